# revision 7
# baseline (speedup 1.0000x reference)
"""DiffMamba cross-attention kernel for 8 Trainium2 NeuronCores.

Problem (hardcoded shapes): B=4, SQ=SK=2048, D=1024, H=16, HD=64.
  q = x @ Wq.T ; k = e @ Wk.T ; v = e @ Wv.T      (per-head split, HD=64)
  out = softmax(q k^T / 8) v                       (merged heads)

Sharding: core c -> (batch b = c//2, head-group hg = c%2).  Each core owns
one batch element and 8 of the 16 heads (rows hg*512:(hg+1)*512 of W), so
all cores are fully independent (no collectives).

Host pre-transposes everything so the device kernel is transpose-free:
  xT [1024,2048], eT [1024,2048], wqT/wkT/wvT [1024,512]  (wqT pre-scaled 1/8)
Device computes outT [512,2048] = (attention output).T; host transposes back.

Device dataflow (bf16 matmul operands, fp32 PSUM accumulate).  The kernel
is ACT-bound: 33.5M exp elements/core at 1 elem/lane/cycle @1.2GHz plus
per-op access overhead is ~284us of ACTIVATE.  Everything else is arranged
to hide under that:

  - Attention steady state, one step per (head-pair p, q-chunk c, SK tile j):
      st[128,1024] = scores for both heads of the pair, via a ROW-TILED
        concurrent matmul pair (contraction 64 at PE rows 0-63 / 64-127).
      pt = exp(st)  (one wide ACT op, bf16 out)
      ctx2[128,512] += va_j^T pt, via a COL-TILED concurrent pair: head A
        -> PSUM partitions 0:64 (tile_position (0,0)), head B -> 64:128
        ((0,64)); both fit ONE psum bank.
      dn[33,512] += ones^T pt: denominators via a second col-tiled pair at
        out partitions 0 and 32 (col groups hold separate 512-row streams).
    Three 512-row PE streams per step (~850ns) < one ACT op (~1.1us).
  - The projections (q/k/v, 48 groups of 8 matmuls each) are SOFTWARE
    PIPELINED into the attention loop: all input DMAs issue up front,
    fine-grained tiles (kt[p][n], qt[p][c] [128,512]; va[j] [128,8,64])
    keep dependencies narrow, and each group is emitted just ahead of its
    first consumer (deadline - LEAD steps), filling the ~300ns/step of PE
    slack.  The old phase-split version serialized ~104us of projections
    before the first exp; here the first exp fires as soon as kt(0,0),
    qt(0,0) land (~15us).
  - Normalization per (p,c): denominator rows -> 32-lane DVE reciprocal
    (cross-partition moves via DMA), GPSIMD partition_broadcast, DVE
    multiply, GPSIMD (SWDGE) output DMA -- all off the ACT/PE queues.
  - A post-pass strips semaphore waits already guaranteed by engine FIFO
    order (tile emits them; walrus allows 1 wait/instruction, so they'd
    otherwise split into ~300ns InstEventSemaphores on the ACT queue).

PSUM budget (8 banks of 2KB/partition): st pool 2x[128,1024]f32 = 4,
ctx2 pool 2x[128,512]f32 = 2, dn pool 1x[33,512]f32 = 1, projection
accumulator 1x[128,512]f32 = 1.
"""

import os
import sys

import numpy as np

_REPO = "/opt/trn_rl_repo"
if os.path.isdir(_REPO) and _REPO not in sys.path:
    sys.path.insert(0, _REPO)

import concourse.bass as bass
import concourse.tile as tile
from concourse import bacc
from concourse import mybir
from concourse.bass_utils import run_bass_kernel_spmd

F32 = mybir.dt.float32
BF16 = mybir.dt.bfloat16
PSUM = bass.MemorySpace.PSUM
EXP = mybir.ActivationFunctionType.Exp

B, S, D = 4, 2048, 1024
DL = 512          # head dims per core (8 heads x 64)
HL = 8            # local heads
NP = 4            # local head pairs
KT = D // 128     # 8 contraction tiles
NCORES = 8
LEAD = 3          # emit projection groups this many steps before first use

_CACHE = {}
LAST_RESULT = None  # BassKernelResults of the most recent run (for profiling)

_DEBUG = os.environ.get("KBG_DEBUG") == "1"


def _strip_dominated_self_waits(nc):
    """Remove semaphore waits that FIFO queue order already guarantees.

    Tile's sem assignment emits a write-after-write wait on the engine's
    OWN tick semaphore for ring-buffer reuse (e.g. each steady-state exp
    waits Activation_* >= k where k counts its own queue's completed
    activations).  Those waits are trivially satisfied -- engine queues
    complete in order -- but they push the instruction to 2 waits, and
    walrus allows only 1, so Bacc splits off an InstEventSemaphore
    (~307ns each on the Scalar queue: 98us of pure overhead in the
    baseline trace).  The upstream optimize_sems pass that would elide
    them is disabled (inc-6505), so do it here: drop any sem-ge-imm wait
    whose target sem (a) is only ever incremented, and (b) has already
    been incremented to >= wait_value by instructions earlier on the
    same engine queue.
    """
    import bass_rust as _br
    from collections import defaultdict

    f = nc.m.functions[0]
    insts = [i for bb in f.blocks for i in bb.instructions]

    never_inc_only = set()  # sem ids with any non-inc update (barriers etc.)
    for inst in insts:
        si = inst.sync_info
        if si is None:
            continue
        for u in si.on_update:
            if u.sync_type == "semaphore" and u.update_mode != "sem-inc":
                never_inc_only.add(u.id)

    inc_same = defaultdict(int)  # (engine, sem_id) -> incs from same queue
    removed = 0
    for inst in insts:
        eng = str(inst.engine)
        si = inst.sync_info
        if si is None:
            continue
        new_waits = []
        changed = False
        for w in si.on_wait:
            if (
                w.sync_type == "semaphore"
                and w.wait_mode == "sem-ge-imm"
                and w.id not in never_inc_only
                and inc_same[(eng, w.id)] >= w.wait_value
            ):
                removed += 1
                changed = True
                continue
            new_waits.append(w)
        if changed:
            inst.sync_info = _br.SyncInfo(
                on_wait=new_waits, on_update=list(si.on_update)
            )
        for u in si.on_update:
            if u.sync_type == "semaphore" and u.update_mode == "sem-inc":
                inc_same[(eng, u.id)] += u.update_value
    return removed


def _build_program():
    # Bacc (not raw Bass): its compile pipeline splits multi-sem waits into
    # EventSemaphore instructions and moves matmul waits onto ldweights --
    # walrus rejects >1 sync wait on most instructions.
    nc = bacc.Bacc()
    xT_h = nc.declare_dram_parameter("xT", [D, S], BF16, isOutput=False)
    eT_h = nc.declare_dram_parameter("eT", [D, S], BF16, isOutput=False)
    wqT_h = nc.declare_dram_parameter("wqT", [D, DL], BF16, isOutput=False)
    wkT_h = nc.declare_dram_parameter("wkT", [D, DL], BF16, isOutput=False)
    wvT_h = nc.declare_dram_parameter("wvT", [D, DL], BF16, isOutput=False)
    outT_h = nc.declare_dram_parameter("outT", [DL, S], F32, isOutput=True)

    # [D, N] viewed as [128, KT, N]: partition p, ktile k -> row k*128+p
    xT_v = xT_h[:].rearrange("(k p) n -> p k n", p=128)
    eT_v = eT_h[:].rearrange("(k p) n -> p k n", p=128)
    wqT_v = wqT_h[:].rearrange("(k p) n -> p k n", p=128)
    wkT_v = wkT_h[:].rearrange("(k p) n -> p k n", p=128)
    wvT_v = wvT_h[:].rearrange("(k p) n -> p k n", p=128)

    with tile.TileContext(nc) as tc:
        with (
            tc.tile_pool(name="persist", bufs=1) as persist,
            tc.tile_pool(name="pp", bufs=1, space=PSUM) as pp,
            tc.tile_pool(name="stp", bufs=2, space=PSUM) as stp,
            tc.tile_pool(name="ctxp", bufs=2, space=PSUM) as ctxp,
            tc.tile_pool(name="dnps", bufs=1, space=PSUM) as dnps,
            tc.tile_pool(name="ptp", bufs=3) as ptp,
            tc.tile_pool(name="dnp", bufs=2) as dnp,
            tc.tile_pool(name="stgp", bufs=4) as stgp,
        ):
            # ---- persistent SBUF state ----
            wk = persist.tile([128, KT, DL], BF16, tag="wk")
            wq = persist.tile([128, KT, DL], BF16, tag="wq")
            wv = persist.tile([128, KT, DL], BF16, tag="wv")
            et = [persist.tile([128, KT, 512], BF16, tag=f"et{n}", name=f"et{n}") for n in range(4)]
            xt = [persist.tile([128, KT, 512], BF16, tag=f"xt{n}", name=f"xt{n}") for n in range(4)]
            # projections, fine-grained so attention steps depend narrowly:
            # kt[p][n]: rows = pair p's 128 head-dims, cols = SK chunk n
            kt = [[persist.tile([128, 512], BF16, tag=f"kt{p}{n}", name=f"kt{p}{n}") for n in range(4)]
                  for p in range(NP)]
            qt = [[persist.tile([128, 512], BF16, tag=f"qt{p}{c}", name=f"qt{p}{c}") for c in range(4)]
                  for p in range(NP)]
            # va[j]: SK tile j, per local head: 64 v-dims (no ones column)
            va = [persist.tile([128, HL, 64], BF16, tag=f"va{j}", name=f"va{j}") for j in range(16)]
            ones1 = persist.tile([128, 1], BF16, tag="ones1")
            zbias = persist.tile([128, 1], F32, tag="zbias")

            nc.vector.memset(zbias[:], 0.0)
            nc.vector.memset(ones1[:], 1.0)

            # ---- all input DMAs up front, first-consumer-first ----
            nc.sync.dma_start(wk[:], wkT_v)
            nc.sync.dma_start(et[0][:], eT_v[:, :, 0:512])
            nc.sync.dma_start(wq[:], wqT_v)
            nc.sync.dma_start(xt[0][:], xT_v[:, :, 0:512])
            nc.sync.dma_start(wv[:], wvT_v)
            for n in range(1, 4):
                nsl = slice(n * 512, (n + 1) * 512)
                nc.sync.dma_start(et[n][:], eT_v[:, :, nsl])
                nc.sync.dma_start(xt[n][:], xT_v[:, :, nsl])

            # ---- projection groups (8 matmuls + 1 cast each) ----
            def do_kt(p, n):
                ps = pp.tile([128, 512], F32, tag="pp")
                msl = slice(p * 128, (p + 1) * 128)
                for k in range(KT):
                    nc.tensor.matmul(ps[:], wk[:, k, msl], et[n][:, k, :],
                                     start=(k == 0), stop=(k == KT - 1))
                nc.vector.tensor_copy(kt[p][n][:], ps[:])

            def do_qt(p, c):
                ps = pp.tile([128, 512], F32, tag="pp")
                msl = slice(p * 128, (p + 1) * 128)
                for k in range(KT):
                    nc.tensor.matmul(ps[:], wq[:, k, msl], xt[c][:, k, :],
                                     start=(k == 0), stop=(k == KT - 1))
                nc.vector.tensor_copy(qt[p][c][:], ps[:])

            def do_va(j):
                n, sub = divmod(j, 4)
                ps = pp.tile([128, 512], F32, tag="pp")
                ssl = slice(sub * 128, (sub + 1) * 128)
                for k in range(KT):
                    nc.tensor.matmul(ps[:], et[n][:, k, ssl], wv[:, k, :],
                                     start=(k == 0), stop=(k == KT - 1))
                nc.vector.tensor_copy(
                    va[j][:], ps[:].rearrange("p (h d) -> p h d", h=HL))

            # deadline (attention step index) -> projection groups to emit
            # just before that step.  step t = (p*4 + c)*16 + j.
            from collections import defaultdict as _dd

            emit_at = _dd(list)
            for p in range(NP):
                for n in range(4):
                    emit_at[max(0, p * 64 + 4 * n - LEAD)].append(("k", p, n))
                for c in range(4):
                    emit_at[max(0, p * 64 + 16 * c - LEAD)].append(("q", p, c))
            for j in range(16):
                emit_at[max(0, j - LEAD)].append(("v", j))

            def emit_groups(step):
                for g in emit_at.pop(step, ()):
                    if g[0] == "k":
                        do_kt(g[1], g[2])
                    elif g[0] == "q":
                        do_qt(g[1], g[2])
                    else:
                        do_va(g[1])

            # ---- attention, software-pipelined with the projections ----
            for p in range(NP):
                for c in range(4):
                    csl = slice(c * 512, (c + 1) * 512)
                    ctx2 = ctxp.tile([128, 512], F32, tag="ctx2")
                    dn = dnps.tile([33, 512], F32, tag="dn")
                    for j in range(16):
                        emit_groups((p * 4 + c) * 16 + j)
                        nj, sub = divmod(j, 4)
                        jsl = slice(sub * 128, (sub + 1) * 128)
                        st = stp.tile([128, 1024], F32, tag="st")
                        # scores: row-tiled concurrent pair (contraction 64)
                        nc.tensor.matmul(st[:, 0:512], kt[p][nj][0:64, jsl],
                                         qt[p][c][0:64, :], start=True, stop=True)
                        nc.tensor.matmul(st[:, 512:1024], kt[p][nj][64:128, jsl],
                                         qt[p][c][64:128, :], start=True, stop=True)
                        pt = ptp.tile([128, 1024], BF16, tag="pt")
                        nc.scalar.activation(pt[:], st[:], EXP, bias=zbias[:, 0:1])
                        # ctx: col-tiled concurrent pair, one psum bank
                        nc.tensor.matmul(ctx2[0:64, :], va[j][:, 2 * p, :],
                                         pt[:, 0:512], start=(j == 0), stop=(j == 15))
                        nc.tensor.matmul(ctx2[64:128, :], va[j][:, 2 * p + 1, :],
                                         pt[:, 512:1024], start=(j == 0), stop=(j == 15))
                        # denominators: col-tiled pair at out partitions 0 / 32
                        nc.tensor.matmul(dn[0:1, :], ones1[:, 0:1],
                                         pt[:, 0:512], start=(j == 0), stop=(j == 15))
                        nc.tensor.matmul(dn[32:33, :], ones1[:, 0:1],
                                         pt[:, 512:1024], start=(j == 0), stop=(j == 15))
                    # stage denominators to SBUF (same-partition DVE copies)
                    dnS = dnp.tile([33, 512], F32, tag="dnS")
                    nc.vector.tensor_copy(dnS[0:1, :], dn[0:1, :])
                    nc.vector.tensor_copy(dnS[32:33, :], dn[32:33, :])
                    # reshape the 1024 denominators across 32 partitions so
                    # the bit-exact reciprocal runs 32 lanes wide; all moves
                    # are DMA (cross-partition), reciprocal on DVE, broadcast
                    # on GPSIMD -- nothing lands on the ACT/PE queues.
                    dnR = dnp.tile([32, 32], F32, tag="dnR")
                    nc.sync.dma_start(dnR[0:16, :], dnS[0:1, :])
                    nc.sync.dma_start(dnR[16:32, :], dnS[32:33, :])
                    rcR = dnp.tile([32, 32], F32, tag="rcR")
                    nc.vector.reciprocal(rcR[:], dnR[:])
                    rc0 = dnp.tile([1, 1024], F32, tag="rc0")
                    nc.sync.dma_start(rc0[:], rcR[:])
                    bcs_a = dnp.tile([64, 512], F32, tag="bcs_a")
                    bcs_b = dnp.tile([64, 512], F32, tag="bcs_b")
                    nc.gpsimd.partition_broadcast(bcs_a[:], rc0[0:1, 0:512])
                    nc.gpsimd.partition_broadcast(bcs_b[:], rc0[0:1, 512:1024])
                    stage_a = stgp.tile([64, 512], F32, tag="stage_a")
                    stage_b = stgp.tile([64, 512], F32, tag="stage_b")
                    nc.vector.tensor_mul(stage_a[:], ctx2[0:64, :], bcs_a[:])
                    nc.vector.tensor_mul(stage_b[:], ctx2[64:128, :], bcs_b[:])
                    # per-chunk output DMA on the idle GPSIMD (SWDGE)
                    nc.gpsimd.dma_start(
                        outT_h[p * 128 : p * 128 + 64, csl], stage_a[:])
                    nc.gpsimd.dma_start(
                        outT_h[p * 128 + 64 : (p + 1) * 128, csl], stage_b[:])

    n = _strip_dominated_self_waits(nc)
    if _DEBUG:
        print(f"stripped {n} dominated self-waits")
    nc.finalize()
    return nc


def kernel(hidden_states, encoder_hidden_states, Wq, Wk, Wv):
    global LAST_RESULT
    hidden_states = np.asarray(hidden_states, dtype=np.float32)
    encoder_hidden_states = np.asarray(encoder_hidden_states, dtype=np.float32)
    Wq = np.asarray(Wq, dtype=np.float32)
    Wk = np.asarray(Wk, dtype=np.float32)
    Wv = np.asarray(Wv, dtype=np.float32)

    if "nc" not in _CACHE:
        _CACHE["nc"] = _build_program()
    nc = _CACHE["nc"]

    import ml_dtypes

    bf16 = ml_dtypes.bfloat16
    in_maps = []
    for c in range(NCORES):
        b, hg = divmod(c, 2)
        rsl = slice(hg * DL, (hg + 1) * DL)
        in_maps.append(
            {
                "xT": np.ascontiguousarray(hidden_states[b].T).astype(bf16),
                "eT": np.ascontiguousarray(encoder_hidden_states[b].T).astype(bf16),
                # fold the 1/sqrt(HD)=1/8 score scale into Wq
                "wqT": np.ascontiguousarray((Wq[rsl] * 0.125).T).astype(bf16),
                "wkT": np.ascontiguousarray(Wk[rsl].T).astype(bf16),
                "wvT": np.ascontiguousarray(Wv[rsl].T).astype(bf16),
            }
        )

    res = run_bass_kernel_spmd(nc, in_maps, list(range(NCORES)))
    LAST_RESULT = res

    out = np.empty((B, S, D), dtype=np.float32)
    for c in range(NCORES):
        b, hg = divmod(c, 2)
        out[b, :, hg * DL : (hg + 1) * DL] = res.results[c]["outT"].T
    return out
